# revision 36
# baseline (speedup 1.0000x reference)
"""MultiHeadAttention with softmax over the HEAD axis — TRN2 Bass kernel.

Reference computation (E=1024, H=16, D=64, B=2, S=2048):
    qh = split_heads(q @ Wq.T); kh = split_heads(k @ Wk.T); vh = split_heads(v @ Wv.T)
    scores = einsum("bhqd,bhkd->bhqk", qh, kh) / 8
    attn = softmax(scores, axis=1)            # over HEADS, not keys
    out = merge_heads(einsum("bhqk,bhkd->bhqd", attn, vh)) @ Wo.T

Sharding: 8 cores = 2 batches x 4 query-blocks of 512. Each core computes
K/V projections for its whole batch (replicated within the 4-core group),
Q projection for its 512 queries, the head-softmax attention, and the output
projection for its query block. No collectives.

Schedule: K/V projection is pipelined into attention pass 0 in 512-key
blocks, so ACT/DVE softmax work overlaps projection matmuls from the start.

On-chip layout (all matmul operands bf16, PSUM f32):
    qhT [e_out, q]  8 tiles [128, 512]   head pair (2t, 2t+1) = partitions
    khT [e_out, j]  8 tiles [128, 2048]    0-63 / 64-127 of tile t
    vh  [j, e_out] 16 tiles [128, 1024]
    scores_T[j, q] per head: K=64 row-tiled matmul (head pair concurrent)
    attn group tile [128, 2jc, 16h, 256q]; softmax over h = exp (ACT) ->
      tree adds + reciprocal_approx_fast + broadcast mul (DVE)
    attnv: out[d-pair 128, q 256] accumulated over j in PSUM, col-tiled pairs
    out projection per pass: outT[e_out, qs:qs+256] = WoT.T @ mergedT

Scheduling notes (hard-won):
  - Concurrent K=64 row-tiled score pairs MUST drain to different PSUM
    banks (same-bank/same-partition concurrent PE writes hang the HW).
  - PE queue is strict FIFO: exp-gated score matmuls are interleaved with
    independent "filler" units (projection / attnv / output projection)
    so the PE never head-of-line blocks; scores, projections and the
    attnv accumulators each get their own PSUM pool.
  - Pass 1 reuses the projection PSUM banks for double-buffered score
    tiles and a 3-deep attn pool; output projection of pass-0 queries is
    hidden inside pass 1.
"""
import os
import sys
sys.path.insert(0, "/opt/trn_rl_repo")

import numpy as np
import ml_dtypes

SCORES_K64 = os.environ.get("SCORES_K64", "1") == "1"
USE_GPS = os.environ.get("USE_GPS", "0") == "1"
_SLOT = (0, 2, 1, 3)  # head dh -> sp/attn_g slot (self-inverse permutation)

import concourse.bass as bass
from concourse import bacc
import concourse.mybir as mybir
import concourse.tile as tile
from concourse.bass_utils import run_bass_kernel_spmd

F32 = mybir.dt.float32
BF16 = mybir.dt.bfloat16
AF = mybir.ActivationFunctionType

B, S, E, H, D = 2, 2048, 1024, 16, 64
NCORES = 8
QB = S * B // NCORES          # 512 queries per core
QP = 256                      # query pass size
NPASS = QB // QP              # 2
SK = S                        # 2048 keys
NJC = SK // 128               # 16 j-chunks
GJC = 2                       # j-chunks per attn group
NGRP = NJC // GJC             # 8 groups
NEI = E // 128                # 8 e_in chunks
NEO = E // 128                # 8 e_out chunks
NBLK = SK // 512              # 4 projection blocks of 512 keys

_CACHED = {}


def build():
    nc = bacc.Bacc(trn_type="TRN2", target_bir_lowering=False)

    qT = nc.dram_tensor("qT", [NEI, 128, QB], BF16, kind="ExternalInput")
    kT = nc.dram_tensor("kT", [NEI, 128, SK], BF16, kind="ExternalInput")
    vT = nc.dram_tensor("vT", [NEI, 128, SK], BF16, kind="ExternalInput")
    WqT = nc.dram_tensor("WqT", [NEI, 128, E], BF16, kind="ExternalInput")
    WkT = nc.dram_tensor("WkT", [NEI, 128, E], BF16, kind="ExternalInput")
    WvT = nc.dram_tensor("WvT", [NEI, 128, E], BF16, kind="ExternalInput")
    WoT = nc.dram_tensor("WoT", [NEI, 128, E], BF16, kind="ExternalInput")
    outT = nc.dram_tensor("outT", [NEO, 128, QB], BF16, kind="ExternalOutput")

    with tile.TileContext(nc) as tc:
        with tc.tile_pool(name="persist", bufs=1) as persist, \
             tc.tile_pool(name="attnp", bufs=2) as attnp, \
             tc.tile_pool(name="t1p", bufs=1) as t1p, \
             tc.tile_pool(name="tmp", bufs=1) as tmp, \
             tc.tile_pool(name="avp", bufs=1, space="PSUM") as avp:

            if SCORES_K64:
                qhT = [persist.tile([128, QB], BF16, tag=f"qhT{i}",
                                    name=f"qhT{i}") for i in range(NEO)]
            else:
                qhTz = [[persist.tile([128, QB], BF16, tag=f"qhTz{par}_{i}",
                                      name=f"qhTz{par}_{i}")
                         for i in range(NEO)] for par in range(2)]
            khT = [persist.tile([128, SK], BF16, tag=f"khT{i}",
                                name=f"khT{i}") for i in range(NEO)]
            vh = [persist.tile([128, E], BF16, tag=f"vh{i}",
                               name=f"vh{i}") for i in range(NJC)]
            mergedT = [persist.tile([128, QB], BF16, tag=f"mergedT{i}",
                                    name=f"mergedT{i}") for i in range(NEO)]

            def scores_softmax(p, g, spool, filler=None, apool=None):
                qs = p * QP
                attn_g = (apool or attnp).tile([128, GJC, H, QP], BF16,
                                               tag="attn", name=f"attn{p}_{g}")
                for jj in range(GJC):
                    jc = g * GJC + jj
                    for hg in range(H // 4):
                        sp = spool.tile([128, 4, QP], F32, tag="sp")
                        for dh in range(4):
                            h = hg * 4 + dh
                            t, par = h // 2, h % 2
                            if SCORES_K64:
                                # Row-tiled K=64 head pairs run concurrently
                                # on the PE; pair members must drain to
                                # DIFFERENT PSUM banks -> head h goes to
                                # slot [0,2,1,3][dh] (par=0 -> bank 0,
                                # par=1 -> bank 1). attn_g holds heads in
                                # this permuted slot order (softmax over h
                                # is order-invariant; attnv un-permutes).
                                nc.tensor.matmul(
                                    sp[:, _SLOT[dh], :],
                                    khT[t][par * 64:(par + 1) * 64,
                                           jc * 128:(jc + 1) * 128],
                                    qhT[t][par * 64:(par + 1) * 64,
                                           qs:qs + QP],
                                    start=True, stop=True)
                            else:
                                nc.tensor.matmul(
                                    sp[:, dh, :],
                                    khT[t][:, jc * 128:(jc + 1) * 128],
                                    qhTz[par][t][:, qs:qs + QP],
                                    start=True, stop=True)
                        nc.scalar.activation(
                            attn_g[:, jj, hg * 4:(hg + 1) * 4, :],
                            sp, AF.Exp, scale=0.125)
                        if filler is not None:
                            filler()
                # softmax over h: tree-sum, fast reciprocal. In pass 1
                # DVE is the throughput bottleneck; GPSIMD (idle, ~3.5x
                # slower) takes t1 for alternating groups — the extra chain
                # latency pipelines away with the 3-deep attn pool.
                t1 = t1p.tile([128, GJC, 8, QP], BF16, tag="t1")
                nc.vector.tensor_add(t1, attn_g[:, :, 0:8, :],
                                     attn_g[:, :, 8:16, :])
                t2 = tmp.tile([128, GJC, 4, QP], BF16, tag="t2")
                nc.vector.tensor_add(t2, t1[:, :, 0:4, :], t1[:, :, 4:8, :])
                t3 = tmp.tile([128, GJC, 2, QP], BF16, tag="t3")
                nc.vector.tensor_add(t3, t2[:, :, 0:2, :], t2[:, :, 2:4, :])
                zf = tmp.tile([128, GJC, QP], F32, tag="zf")
                nc.vector.tensor_add(zf, t3[:, :, 0, :], t3[:, :, 1, :])
                rf = tmp.tile([128, GJC, QP], F32, tag="rf")
                nc.vector.reciprocal_approx_fast(
                    out=rf.rearrange("p a q -> p (a q)"),
                    in_=zf.rearrange("p a q -> p (a q)"))
                r16 = tmp.tile([128, GJC, QP], BF16, tag="r16")
                eng2 = nc.gpsimd if USE_GPS else nc.vector
                eng2.tensor_copy(r16, rf)
                nc.vector.tensor_mul(
                    attn_g, attn_g,
                    r16.unsqueeze(2).broadcast_to([128, GJC, H, QP]))
                return attn_g

            def attnv(avt, attn_g, g):
                for jj in range(GJC):
                    jc = g * GJC + jj
                    for h in range(H):
                        # slot in attn_g holding head h (see scores_softmax)
                        m = (h // 4) * 4 + _SLOT[h % 4] if SCORES_K64 else h
                        pp, half = h // 2, (h % 2) * 64
                        nc.tensor.matmul(
                            avt[pp // 2][half:half + 64, pp % 2, :],
                            vh[jc][:, h * 64:(h + 1) * 64],
                            attn_g[:, jj, m, :],
                            start=(jc == 0 and h % 4 < 2),
                            stop=(jc == NJC - 1 and h % 4 >= 2),
                            skip_group_check=True)

            def attnv_units(avt, attn_g, g):
                us = []
                for jj in range(GJC):
                    jc = g * GJC + jj
                    for hq in range(H // 4):
                        def unit(jj=jj, jc=jc, hq=hq, attn_g=attn_g):
                            for dh in range(4):
                                h = hq * 4 + dh
                                m = ((h // 4) * 4 + _SLOT[h % 4]
                                     if SCORES_K64 else h)
                                pp, half = h // 2, (h % 2) * 64
                                nc.tensor.matmul(
                                    avt[pp // 2][half:half + 64, pp % 2, :],
                                    vh[jc][:, h * 64:(h + 1) * 64],
                                    attn_g[:, jj, m, :],
                                    start=(jc == 0 and h % 4 < 2),
                                    stop=(jc == NJC - 1 and h % 4 >= 2),
                                    skip_group_check=True)
                        us.append(unit)
                return us

            def merge(avt, p):
                qs = p * QP
                for pp in range(8):
                    nc.scalar.copy(mergedT[pp][:, qs:qs + QP],
                                   avt[pp // 2][:, pp % 2, :])

            # ------------- pass 0: projections pipelined with attention ----
            with tc.tile_pool(name="pwk", bufs=1) as pwk, \
                 tc.tile_pool(name="pwv", bufs=1) as pwv, \
                 tc.tile_pool(name="pxkv", bufs=3) as pxkv, \
                 tc.tile_pool(name="spp", bufs=1, space="PSUM") as spp, \
                 tc.tile_pool(name="pjp", bufs=2, space="PSUM") as pjp:

                w_k = [pwk.tile([128, E], BF16, tag=f"wk{i}",
                                name=f"wk{i}") for i in range(NEI)]
                w_v = [pwv.tile([128, E], BF16, tag=f"wv{i}",
                                name=f"wv{i}") for i in range(NEI)]

                xk_t, xv_t = {}, {}

                def stage_k(jn):
                    xk = [pxkv.tile([128, 512], BF16, tag=f"xkv{i}",
                                    name=f"xk{jn}_{i}") for i in range(NEI)]
                    for i in range(NEI):
                        nc.sync.dma_start(out=xk[i],
                                          in_=kT[i][:, jn * 512:(jn + 1) * 512])
                    xk_t[jn] = xk

                def stage_v(jn):
                    xv = [pxkv.tile([128, 512], BF16, tag=f"xkv{i}",
                                    name=f"xv{jn}_{i}") for i in range(NEI)]
                    for i in range(NEI):
                        nc.sync.dma_start(out=xv[i],
                                          in_=vT[i][:, jn * 512:(jn + 1) * 512])
                    xv_t[jn] = xv

                def kproj_unit(jn, eo):
                    xk = xk_t[jn]
                    if True:
                        ps = pjp.tile([128, 512], F32, tag="pj")
                        for ki in range(NEI):
                            nc.tensor.matmul(
                                ps,
                                w_k[ki][:, eo * 128:(eo + 1) * 128],
                                xk[ki],
                                start=(ki == 0), stop=(ki == NEI - 1))
                        dst = khT[eo][:, jn * 512:(jn + 1) * 512]
                        if eo % 2 == 0:
                            nc.scalar.copy(dst, ps)
                        else:
                            nc.vector.tensor_copy(dst, ps)

                def vproj_unit(jn, jj, en):
                    xv = xv_t[jn]
                    if True:
                        jc = jn * 4 + jj
                        if True:
                            ps = pjp.tile([128, 512], F32, tag="pj")
                            for ki in range(NEI):
                                nc.tensor.matmul(
                                    ps,
                                    xv[ki][:, jj * 128:(jj + 1) * 128],
                                    w_v[ki][:, en * 512:(en + 1) * 512],
                                    start=(ki == 0), stop=(ki == NEI - 1))
                            if jj % 2 == 0:
                                nc.scalar.copy(
                                    vh[jc][:, en * 512:(en + 1) * 512], ps)
                            else:
                                nc.vector.tensor_copy(
                                    vh[jc][:, en * 512:(en + 1) * 512], ps)

                # Q projection first (own queries only)
                with tc.tile_pool(name="pwq", bufs=1) as pwq:
                    w_q = [pwq.tile([128, E], BF16, tag=f"wq{i}",
                                    name=f"wq{i}") for i in range(NEI)]
                    x_q = [pxkv.tile([128, QB], BF16, tag=f"xkv{i}",
                                     name=f"xq{i}") for i in range(NEI)]
                    for i in range(NEI):
                        nc.sync.dma_start(out=x_q[i], in_=qT[i])
                    # first halves of w_q cover Qproj eo 0-3 -> PE starts
                    # after ~2MB of DMA instead of 3MB
                    for i in range(NEI):
                        nc.sync.dma_start(out=w_q[i][:, 0:512],
                                          in_=WqT[i][:, 0:512])
                    for i in range(NEI):
                        nc.sync.dma_start(out=w_q[i][:, 512:E],
                                          in_=WqT[i][:, 512:E])
                    for i in range(NEI):
                        nc.sync.dma_start(out=w_k[i], in_=WkT[i])
                    stage_k(0)
                    for i in range(NEI):
                        nc.sync.dma_start(out=w_v[i], in_=WvT[i])

                    if not SCORES_K64:
                        for par in range(2):
                            for eo in range(NEO):
                                nc.vector.memset(qhTz[par][eo], 0.0)
                    for eo in range(NEO):
                        ps = pjp.tile([128, 512], F32, tag="pj")
                        for ki in range(NEI):
                            nc.tensor.matmul(
                                ps,
                                w_q[ki][:, eo * 128:(eo + 1) * 128],
                                x_q[ki],
                                start=(ki == 0), stop=(ki == NEI - 1))
                        if SCORES_K64:
                            nc.scalar.copy(qhT[eo], ps)
                        else:
                            nc.scalar.copy(qhTz[0][eo][0:64, :], ps[0:64, :])
                            nc.scalar.copy(qhTz[1][eo][64:128, :],
                                           ps[64:128, :])

                for eo in range(NEO):
                    kproj_unit(0, eo)
                stage_k(1)
                stage_v(0)

                from collections import deque
                units = deque()

                def filler():
                    n = 2 if len(units) > 8 else 1
                    for _ in range(n):
                        if units:
                            units.popleft()()

                avt = [avp.tile([128, 2, QP], F32, tag=f"avt{i}",
                                name=f"avt0_{i}") for i in range(4)]
                for g in range(NGRP):
                    if g % 2 == 0:
                        # vproj(jn) units first: they must drain before
                        # attnv(2jn) is emitted (same PE FIFO) — attnv
                        # waits on vh copies.
                        jn = g // 2
                        for jj in range(4):
                            for en in range(E // 512):
                                units.append(
                                    lambda jn=jn, jj=jj, en=en:
                                    vproj_unit(jn, jj, en))
                        if jn + 1 < NBLK:
                            for eo in range(NEO):
                                units.append(
                                    lambda jn=jn, eo=eo: kproj_unit(jn + 1, eo))
                        if jn + 2 < NBLK:
                            stage_k(jn + 2)
                        if jn + 1 < NBLK:
                            stage_v(jn + 1)
                    attn_g = scores_softmax(0, g, spp, filler)
                    units.extend(attnv_units(avt, attn_g, g))
                while units:
                    units.popleft()()
                merge(avt, 0)

            # ------------- pass 1 + output projections ---------------------
            with tc.tile_pool(name="pwo", bufs=1) as pwo, \
                 tc.tile_pool(name="outp", bufs=4) as outp, \
                 tc.tile_pool(name="attnp2", bufs=3) as attnp2, \
                 tc.tile_pool(name="sp2", bufs=2, space="PSUM") as sp2:
                w_o = [pwo.tile([128, E], BF16, tag=f"wo{i}",
                                name=f"wo{i}") for i in range(NEI)]
                for i in range(NEI):
                    nc.sync.dma_start(out=w_o[i], in_=WoT[i])

                def outproj_unit(p, eo):
                    qs = p * QP
                    ps = sp2.tile([128, QP], F32, tag="sp")
                    for ki in range(NEI):
                        nc.tensor.matmul(
                            ps,
                            w_o[ki][:, eo * 128:(eo + 1) * 128],
                            mergedT[ki][:, qs:qs + QP],
                            start=(ki == 0), stop=(ki == NEI - 1))
                    ot = outp.tile([128, QP], BF16, tag="ot")
                    nc.scalar.copy(ot, ps)
                    nc.sync.dma_start(out=outT[eo][:, qs:qs + QP], in_=ot)

                def outproj(p):
                    for eo in range(NEO):
                        outproj_unit(p, eo)

                avt = [avp.tile([128, 2, QP], F32, tag=f"avt{i}",
                                name=f"avt1_{i}") for i in range(4)]
                from collections import deque
                units1 = deque()

                def filler1():
                    n = 2 if len(units1) > 8 else 1
                    for _ in range(n):
                        if units1:
                            units1.popleft()()

                for g in range(NGRP):
                    attn_g = scores_softmax(1, g, sp2, filler1, attnp2)
                    units1.extend(attnv_units(avt, attn_g, g))
                    if g == 0:
                        units1.extend(
                            lambda eo=eo: outproj_unit(0, eo)
                            for eo in range(NEO))
                while units1:
                    units1.popleft()()
                merge(avt, 1)
                outproj(1)

    nc.compile()
    return nc


def _get_nc():
    if "nc" not in _CACHED:
        _CACHED["nc"] = build()
    return _CACHED["nc"]


def kernel(q, k, v, Wq, Wk, Wv, Wo, **unused):
    q = np.asarray(q, dtype=np.float32)
    k = np.asarray(k, dtype=np.float32)
    v = np.asarray(v, dtype=np.float32)

    bf = ml_dtypes.bfloat16

    def prep_w(W):
        # [out,in] -> W.T [in,out] -> [NEI, 128, E], bf16
        return np.ascontiguousarray(
            np.asarray(W, dtype=np.float32).T.reshape(NEI, 128, E)
        ).astype(bf)

    WqT, WkT, WvT, WoT = map(prep_w, (Wq, Wk, Wv, Wo))

    kT_b, vT_b = [], []
    for b in range(B):
        kT_b.append(np.ascontiguousarray(
            k[b].T.reshape(NEI, 128, SK)).astype(bf))
        vT_b.append(np.ascontiguousarray(
            v[b].T.reshape(NEI, 128, SK)).astype(bf))

    in_maps = []
    for c in range(NCORES):
        b, qs = c // 4, (c % 4) * QB
        qT_c = np.ascontiguousarray(
            q[b].T[:, qs:qs + QB].reshape(NEI, 128, QB)).astype(bf)
        in_maps.append({
            "qT": qT_c, "kT": kT_b[b], "vT": vT_b[b],
            "WqT": WqT, "WkT": WkT, "WvT": WvT, "WoT": WoT,
        })

    nc = _get_nc()
    res = run_bass_kernel_spmd(nc, in_maps, core_ids=list(range(NCORES)))

    out = np.empty((B, S, E), dtype=np.float32)
    for c in range(NCORES):
        b, qs = c // 4, (c % 4) * QB
        oT = res.results[c]["outT"].reshape(E, QB).astype(np.float32)
        out[b, qs:qs + QB, :] = oT.T
    return out


# revision 37
# speedup vs baseline: 1.0146x; 1.0146x over previous
"""MultiHeadAttention with softmax over the HEAD axis — TRN2 Bass kernel.

Reference computation (E=1024, H=16, D=64, B=2, S=2048):
    qh = split_heads(q @ Wq.T); kh = split_heads(k @ Wk.T); vh = split_heads(v @ Wv.T)
    scores = einsum("bhqd,bhkd->bhqk", qh, kh) / 8
    attn = softmax(scores, axis=1)            # over HEADS, not keys
    out = merge_heads(einsum("bhqk,bhkd->bhqd", attn, vh)) @ Wo.T

Sharding: 8 cores = 2 batches x 4 query-blocks of 512. Each core computes
K/V projections for its whole batch (replicated within the 4-core group),
Q projection for its 512 queries, the head-softmax attention, and the output
projection for its query block. No collectives.

Schedule: K/V projection is pipelined into attention pass 0 in 512-key
blocks, so ACT/DVE softmax work overlaps projection matmuls from the start.

On-chip layout (all matmul operands bf16, PSUM f32):
    qhT [e_out, q]  8 tiles [128, 512]   head pair (2t, 2t+1) = partitions
    khT [e_out, j]  8 tiles [128, 2048]    0-63 / 64-127 of tile t
    vh  [j, e_out] 16 tiles [128, 1024]
    scores_T[j, q] per head: K=64 row-tiled matmul (head pair concurrent)
    attn group tile [128, 2jc, 16h, 256q]; softmax over h = exp (ACT) ->
      tree adds + reciprocal_approx_fast + broadcast mul (DVE)
    attnv: out[d-pair 128, q 256] accumulated over j in PSUM, col-tiled pairs
    out projection per pass: outT[e_out, qs:qs+256] = WoT.T @ mergedT

Scheduling notes (hard-won):
  - Concurrent K=64 row-tiled score pairs MUST drain to different PSUM
    banks (same-bank/same-partition concurrent PE writes hang the HW).
  - PE queue is strict FIFO: exp-gated score matmuls are interleaved with
    independent "filler" units (projection / attnv / output projection)
    so the PE never head-of-line blocks; scores, projections and the
    attnv accumulators each get their own PSUM pool.
  - Pass 1 reuses the projection PSUM banks for double-buffered score
    tiles and a 3-deep attn pool; output projection of pass-0 queries is
    hidden inside pass 1.
"""
import os
import sys
sys.path.insert(0, "/opt/trn_rl_repo")

import numpy as np
import ml_dtypes

SCORES_K64 = os.environ.get("SCORES_K64", "1") == "1"
USE_GPS = os.environ.get("USE_GPS", "0") == "1"
_SLOT = (0, 2, 1, 3)  # head dh -> sp/attn_g slot (self-inverse permutation)

import concourse.bass as bass
from concourse import bacc
import concourse.mybir as mybir
import concourse.tile as tile
from concourse.bass_utils import run_bass_kernel_spmd

F32 = mybir.dt.float32
BF16 = mybir.dt.bfloat16
AF = mybir.ActivationFunctionType

B, S, E, H, D = 2, 2048, 1024, 16, 64
NCORES = 8
QB = S * B // NCORES          # 512 queries per core
QP = 256                      # query pass size
NPASS = QB // QP              # 2
SK = S                        # 2048 keys
NJC = SK // 128               # 16 j-chunks
GJC = 2                       # j-chunks per attn group
NGRP = NJC // GJC             # 8 groups
NEI = E // 128                # 8 e_in chunks
NEO = E // 128                # 8 e_out chunks
NBLK = SK // 512              # 4 projection blocks of 512 keys

_CACHED = {}


def build():
    nc = bacc.Bacc(trn_type="TRN2", target_bir_lowering=False)

    qT = nc.dram_tensor("qT", [NEI, 128, QB], BF16, kind="ExternalInput")
    kT = nc.dram_tensor("kT", [NEI, 128, SK], BF16, kind="ExternalInput")
    vT = nc.dram_tensor("vT", [NEI, 128, SK], BF16, kind="ExternalInput")
    WqT = nc.dram_tensor("WqT", [NEI, 128, E], BF16, kind="ExternalInput")
    WkT = nc.dram_tensor("WkT", [NEI, 128, E], BF16, kind="ExternalInput")
    WvT = nc.dram_tensor("WvT", [NEI, 128, E], BF16, kind="ExternalInput")
    WoT = nc.dram_tensor("WoT", [NEI, 128, E], BF16, kind="ExternalInput")
    outT = nc.dram_tensor("outT", [NEO, 128, QB], BF16, kind="ExternalOutput")

    with tile.TileContext(nc) as tc:
        with tc.tile_pool(name="persist", bufs=1) as persist, \
             tc.tile_pool(name="attnp", bufs=2) as attnp, \
             tc.tile_pool(name="t1p", bufs=1) as t1p, \
             tc.tile_pool(name="tmp", bufs=1) as tmp, \
             tc.tile_pool(name="avp", bufs=1, space="PSUM") as avp:

            if SCORES_K64:
                qhT = [persist.tile([128, QB], BF16, tag=f"qhT{i}",
                                    name=f"qhT{i}") for i in range(NEO)]
            else:
                qhTz = [[persist.tile([128, QB], BF16, tag=f"qhTz{par}_{i}",
                                      name=f"qhTz{par}_{i}")
                         for i in range(NEO)] for par in range(2)]
            khT = [persist.tile([128, SK], BF16, tag=f"khT{i}",
                                name=f"khT{i}") for i in range(NEO)]
            vh = [persist.tile([128, E], BF16, tag=f"vh{i}",
                               name=f"vh{i}") for i in range(NJC)]
            mergedT = [persist.tile([128, QB], BF16, tag=f"mergedT{i}",
                                    name=f"mergedT{i}") for i in range(NEO)]

            def scores_softmax(p, g, spool, filler=None, apool=None):
                qs = p * QP
                attn_g = (apool or attnp).tile([128, GJC, H, QP], BF16,
                                               tag="attn", name=f"attn{p}_{g}")
                for jj in range(GJC):
                    jc = g * GJC + jj
                    for hg in range(H // 4):
                        sp = spool.tile([128, 4, QP], F32, tag="sp")
                        for dh in range(4):
                            h = hg * 4 + dh
                            t, par = h // 2, h % 2
                            if SCORES_K64:
                                # Row-tiled K=64 head pairs run concurrently
                                # on the PE; pair members must drain to
                                # DIFFERENT PSUM banks -> head h goes to
                                # slot [0,2,1,3][dh] (par=0 -> bank 0,
                                # par=1 -> bank 1). attn_g holds heads in
                                # this permuted slot order (softmax over h
                                # is order-invariant; attnv un-permutes).
                                nc.tensor.matmul(
                                    sp[:, _SLOT[dh], :],
                                    khT[t][par * 64:(par + 1) * 64,
                                           jc * 128:(jc + 1) * 128],
                                    qhT[t][par * 64:(par + 1) * 64,
                                           qs:qs + QP],
                                    start=True, stop=True)
                            else:
                                nc.tensor.matmul(
                                    sp[:, dh, :],
                                    khT[t][:, jc * 128:(jc + 1) * 128],
                                    qhTz[par][t][:, qs:qs + QP],
                                    start=True, stop=True)
                        nc.scalar.activation(
                            attn_g[:, jj, hg * 4:(hg + 1) * 4, :],
                            sp, AF.Exp, scale=0.125)
                        if filler is not None:
                            filler()
                # softmax over h: tree-sum, fast reciprocal. In pass 1
                # DVE is the throughput bottleneck; GPSIMD (idle, ~3.5x
                # slower) takes t1 for alternating groups — the extra chain
                # latency pipelines away with the 3-deep attn pool.
                t1 = t1p.tile([128, GJC, 8, QP], BF16, tag="t1")
                nc.vector.tensor_add(t1, attn_g[:, :, 0:8, :],
                                     attn_g[:, :, 8:16, :])
                t2 = tmp.tile([128, GJC, 4, QP], BF16, tag="t2")
                nc.vector.tensor_add(t2, t1[:, :, 0:4, :], t1[:, :, 4:8, :])
                t3 = tmp.tile([128, GJC, 2, QP], BF16, tag="t3")
                nc.vector.tensor_add(t3, t2[:, :, 0:2, :], t2[:, :, 2:4, :])
                zf = tmp.tile([128, GJC, QP], F32, tag="zf")
                nc.vector.tensor_add(zf, t3[:, :, 0, :], t3[:, :, 1, :])
                rf = tmp.tile([128, GJC, QP], F32, tag="rf")
                nc.vector.reciprocal_approx_fast(
                    out=rf.rearrange("p a q -> p (a q)"),
                    in_=zf.rearrange("p a q -> p (a q)"))
                r16 = tmp.tile([128, GJC, QP], BF16, tag="r16")
                eng2 = nc.gpsimd if USE_GPS else nc.vector
                eng2.tensor_copy(r16, rf)
                nc.vector.tensor_mul(
                    attn_g, attn_g,
                    r16.unsqueeze(2).broadcast_to([128, GJC, H, QP]))
                return attn_g

            def attnv(avt, attn_g, g):
                for jj in range(GJC):
                    jc = g * GJC + jj
                    for h in range(H):
                        # slot in attn_g holding head h (see scores_softmax)
                        m = (h // 4) * 4 + _SLOT[h % 4] if SCORES_K64 else h
                        pp, half = h // 2, (h % 2) * 64
                        nc.tensor.matmul(
                            avt[pp // 2][half:half + 64, pp % 2, :],
                            vh[jc][:, h * 64:(h + 1) * 64],
                            attn_g[:, jj, m, :],
                            start=(jc == 0 and h % 4 < 2),
                            stop=(jc == NJC - 1 and h % 4 >= 2),
                            skip_group_check=True)

            def attnv_units(avt, attn_g, g):
                us = []
                for jj in range(GJC):
                    jc = g * GJC + jj
                    for hq in range(H // 4):
                        def unit(jj=jj, jc=jc, hq=hq, attn_g=attn_g):
                            for dh in range(4):
                                h = hq * 4 + dh
                                m = ((h // 4) * 4 + _SLOT[h % 4]
                                     if SCORES_K64 else h)
                                pp, half = h // 2, (h % 2) * 64
                                nc.tensor.matmul(
                                    avt[pp // 2][half:half + 64, pp % 2, :],
                                    vh[jc][:, h * 64:(h + 1) * 64],
                                    attn_g[:, jj, m, :],
                                    start=(jc == 0 and h % 4 < 2),
                                    stop=(jc == NJC - 1 and h % 4 >= 2),
                                    skip_group_check=True)
                        us.append(unit)
                return us

            def merge(avt, p):
                qs = p * QP
                for pp in range(8):
                    nc.scalar.copy(mergedT[pp][:, qs:qs + QP],
                                   avt[pp // 2][:, pp % 2, :])

            # ------------- pass 0: projections pipelined with attention ----
            with tc.tile_pool(name="pwk", bufs=1) as pwk, \
                 tc.tile_pool(name="pwv", bufs=1) as pwv, \
                 tc.tile_pool(name="pxkv", bufs=3) as pxkv, \
                 tc.tile_pool(name="spp", bufs=1, space="PSUM") as spp, \
                 tc.tile_pool(name="pjp", bufs=2, space="PSUM") as pjp:

                w_k = [pwk.tile([128, E], BF16, tag=f"wk{i}",
                                name=f"wk{i}") for i in range(NEI)]
                w_v = [pwv.tile([128, E], BF16, tag=f"wv{i}",
                                name=f"wv{i}") for i in range(NEI)]

                xk_t, xv_t = {}, {}

                def stage_k(jn):
                    xk = [pxkv.tile([128, 512], BF16, tag=f"xkv{i}",
                                    name=f"xk{jn}_{i}") for i in range(NEI)]
                    for i in range(NEI):
                        nc.sync.dma_start(out=xk[i],
                                          in_=kT[i][:, jn * 512:(jn + 1) * 512])
                    xk_t[jn] = xk

                def stage_v(jn):
                    xv = [pxkv.tile([128, 512], BF16, tag=f"xkv{i}",
                                    name=f"xv{jn}_{i}") for i in range(NEI)]
                    for i in range(NEI):
                        nc.sync.dma_start(out=xv[i],
                                          in_=vT[i][:, jn * 512:(jn + 1) * 512])
                    xv_t[jn] = xv

                def kproj_unit(jn, eo):
                    xk = xk_t[jn]
                    if True:
                        ps = pjp.tile([128, 512], F32, tag="pj")
                        for ki in range(NEI):
                            nc.tensor.matmul(
                                ps,
                                w_k[ki][:, eo * 128:(eo + 1) * 128],
                                xk[ki],
                                start=(ki == 0), stop=(ki == NEI - 1))
                        dst = khT[eo][:, jn * 512:(jn + 1) * 512]
                        if eo % 2 == 0:
                            nc.scalar.copy(dst, ps)
                        else:
                            nc.vector.tensor_copy(dst, ps)

                def vproj_unit(jn, jj, en):
                    xv = xv_t[jn]
                    if True:
                        jc = jn * 4 + jj
                        if True:
                            ps = pjp.tile([128, 512], F32, tag="pj")
                            for ki in range(NEI):
                                nc.tensor.matmul(
                                    ps,
                                    xv[ki][:, jj * 128:(jj + 1) * 128],
                                    w_v[ki][:, en * 512:(en + 1) * 512],
                                    start=(ki == 0), stop=(ki == NEI - 1))
                            if jj % 2 == 0:
                                nc.scalar.copy(
                                    vh[jc][:, en * 512:(en + 1) * 512], ps)
                            else:
                                nc.vector.tensor_copy(
                                    vh[jc][:, en * 512:(en + 1) * 512], ps)

                # Q projection first (own queries only)
                with tc.tile_pool(name="pwq", bufs=1) as pwq:
                    w_q = [pwq.tile([128, E], BF16, tag=f"wq{i}",
                                    name=f"wq{i}") for i in range(NEI)]
                    x_q = [pxkv.tile([128, QB], BF16, tag=f"xkv{i}",
                                     name=f"xq{i}") for i in range(NEI)]
                    for i in range(NEI):
                        nc.sync.dma_start(out=x_q[i], in_=qT[i])
                        nc.sync.dma_start(out=w_q[i], in_=WqT[i])
                    for i in range(NEI):
                        nc.sync.dma_start(out=w_k[i], in_=WkT[i])
                    stage_k(0)
                    for i in range(NEI):
                        nc.sync.dma_start(out=w_v[i], in_=WvT[i])

                    if not SCORES_K64:
                        for par in range(2):
                            for eo in range(NEO):
                                nc.vector.memset(qhTz[par][eo], 0.0)
                    for eo in range(NEO):
                        ps = pjp.tile([128, 512], F32, tag="pj")
                        for ki in range(NEI):
                            nc.tensor.matmul(
                                ps,
                                w_q[ki][:, eo * 128:(eo + 1) * 128],
                                x_q[ki],
                                start=(ki == 0), stop=(ki == NEI - 1))
                        if SCORES_K64:
                            nc.scalar.copy(qhT[eo], ps)
                        else:
                            nc.scalar.copy(qhTz[0][eo][0:64, :], ps[0:64, :])
                            nc.scalar.copy(qhTz[1][eo][64:128, :],
                                           ps[64:128, :])

                for eo in range(NEO):
                    kproj_unit(0, eo)
                stage_k(1)
                stage_v(0)

                from collections import deque
                units = deque()

                def filler():
                    n = 2 if len(units) > 8 else 1
                    for _ in range(n):
                        if units:
                            units.popleft()()

                avt = [avp.tile([128, 2, QP], F32, tag=f"avt{i}",
                                name=f"avt0_{i}") for i in range(4)]
                for g in range(NGRP):
                    if g % 2 == 0:
                        # vproj(jn) units first: they must drain before
                        # attnv(2jn) is emitted (same PE FIFO) — attnv
                        # waits on vh copies.
                        jn = g // 2
                        for jj in range(4):
                            for en in range(E // 512):
                                units.append(
                                    lambda jn=jn, jj=jj, en=en:
                                    vproj_unit(jn, jj, en))
                        if jn + 1 < NBLK:
                            for eo in range(NEO):
                                units.append(
                                    lambda jn=jn, eo=eo: kproj_unit(jn + 1, eo))
                        if jn + 2 < NBLK:
                            stage_k(jn + 2)
                        if jn + 1 < NBLK:
                            stage_v(jn + 1)
                    attn_g = scores_softmax(0, g, spp, filler)
                    units.extend(attnv_units(avt, attn_g, g))
                while units:
                    units.popleft()()
                merge(avt, 0)

            # ------------- pass 1 + output projections ---------------------
            with tc.tile_pool(name="pwo", bufs=1) as pwo, \
                 tc.tile_pool(name="outp", bufs=4) as outp, \
                 tc.tile_pool(name="attnp2", bufs=3) as attnp2, \
                 tc.tile_pool(name="sp2", bufs=2, space="PSUM") as sp2:
                w_o = [pwo.tile([128, E], BF16, tag=f"wo{i}",
                                name=f"wo{i}") for i in range(NEI)]
                for i in range(NEI):
                    nc.sync.dma_start(out=w_o[i], in_=WoT[i])

                def outproj_unit(p, eo):
                    qs = p * QP
                    ps = sp2.tile([128, QP], F32, tag="sp")
                    for ki in range(NEI):
                        nc.tensor.matmul(
                            ps,
                            w_o[ki][:, eo * 128:(eo + 1) * 128],
                            mergedT[ki][:, qs:qs + QP],
                            start=(ki == 0), stop=(ki == NEI - 1))
                    ot = outp.tile([128, QP], BF16, tag="ot")
                    nc.scalar.copy(ot, ps)
                    nc.sync.dma_start(out=outT[eo][:, qs:qs + QP], in_=ot)

                def outproj(p):
                    for eo in range(NEO):
                        outproj_unit(p, eo)

                avt = [avp.tile([128, 2, QP], F32, tag=f"avt{i}",
                                name=f"avt1_{i}") for i in range(4)]
                from collections import deque
                units1 = deque()

                def filler1():
                    n = 2 if len(units1) > 8 else 1
                    for _ in range(n):
                        if units1:
                            units1.popleft()()

                for g in range(NGRP):
                    attn_g = scores_softmax(1, g, sp2, filler1, attnp2)
                    units1.extend(attnv_units(avt, attn_g, g))
                    if g == 0:
                        units1.extend(
                            lambda eo=eo: outproj_unit(0, eo)
                            for eo in range(NEO))
                while units1:
                    units1.popleft()()
                merge(avt, 1)
                outproj(1)

    nc.compile()
    return nc


def _get_nc():
    if "nc" not in _CACHED:
        _CACHED["nc"] = build()
    return _CACHED["nc"]


def kernel(q, k, v, Wq, Wk, Wv, Wo, **unused):
    q = np.asarray(q, dtype=np.float32)
    k = np.asarray(k, dtype=np.float32)
    v = np.asarray(v, dtype=np.float32)

    bf = ml_dtypes.bfloat16

    def prep_w(W):
        # [out,in] -> W.T [in,out] -> [NEI, 128, E], bf16
        return np.ascontiguousarray(
            np.asarray(W, dtype=np.float32).T.reshape(NEI, 128, E)
        ).astype(bf)

    WqT, WkT, WvT, WoT = map(prep_w, (Wq, Wk, Wv, Wo))

    kT_b, vT_b = [], []
    for b in range(B):
        kT_b.append(np.ascontiguousarray(
            k[b].T.reshape(NEI, 128, SK)).astype(bf))
        vT_b.append(np.ascontiguousarray(
            v[b].T.reshape(NEI, 128, SK)).astype(bf))

    in_maps = []
    for c in range(NCORES):
        b, qs = c // 4, (c % 4) * QB
        qT_c = np.ascontiguousarray(
            q[b].T[:, qs:qs + QB].reshape(NEI, 128, QB)).astype(bf)
        in_maps.append({
            "qT": qT_c, "kT": kT_b[b], "vT": vT_b[b],
            "WqT": WqT, "WkT": WkT, "WvT": WvT, "WoT": WoT,
        })

    nc = _get_nc()
    res = run_bass_kernel_spmd(nc, in_maps, core_ids=list(range(NCORES)))

    out = np.empty((B, S, E), dtype=np.float32)
    for c in range(NCORES):
        b, qs = c // 4, (c % 4) * QB
        oT = res.results[c]["outT"].reshape(E, QB).astype(np.float32)
        out[b, qs:qs + QB, :] = oT.T
    return out
